# revision 48
# baseline (speedup 1.0000x reference)
"""AttentionPerformer Trainium2 kernel.

Strategy: data-parallel over batch B=8 -> one NeuronCore per batch element.
Feature-major (transposed) layout throughout; Performer random-feature
exponent (wtx - 0.5*|k|^2) comes out of the PE via block-diagonal
pair-packed contractions so every elementwise op runs on all 128 partitions.

Math (per b, h; eps folded: dropping both 1/sqrt(M) factors rescales
numerator and denominator by M, so eps becomes M*eps):
    kq^T = W_kq^T @ x^T        (feature-major; head-PAIR blocks [k_h0|k_h1])
    P = kq + b ; B = (kq+b)^2  (full 128-partition ops per pair block)
    e  = blockdiag(w_h0,w_h1)^T P + blockdiag(-.5,-.5)^T B
         (two accumulating matmuls; two pairs pack a [128,T] psum -> 1 EXP
          yields kp/qp for a 4-head group in kptv layout directly)
    kptv[m, n] = sum_t kp[t,m] v[t,n] ;  kp_sum[m] = sum_t kp[t,m]
    G_h = kptv_h @ proj_w_h^T  (fold kptv into proj: [M, C] per head, once)
    D = kp_sum . qp  (packed 4-head matmuls) ; qpn = qp * 1/(D+M*eps)
         (reciprocal broadcast back to 128 partitions via a selector matmul)
    out^T = sum_g G_g^T @ qpn_g + proj_b   (contraction 3x128 instead of 768)

dtypes: fp32r on the exponent-critical path, bf16 on the kp/qp/v/G value
path (errors average over the M=32 contraction).
"""
import sys
sys.path.insert(0, '/opt/trn_rl_repo')

import numpy as np
import ml_dtypes

B, N, C = 8, 4096, 768
H, HD, M = 12, 64, 32
T = 512                 # t-tile size
NT = N // T             # 8 tiles
EPS_EFF = float(M) * 1e-8

_CACHE = {}
TRACE = False
LAST_EXEC_NS = None


def _build():
    import concourse.bass as bass
    import concourse.tile as tile
    from concourse import bacc, mybir

    f32 = mybir.dt.float32
    f32r = mybir.dt.float32r
    bf16 = mybir.dt.bfloat16
    ADD = mybir.AluOpType.add
    MULT = mybir.AluOpType.mult
    EXP = mybir.ActivationFunctionType.Exp
    SQUARE = mybir.ActivationFunctionType.Square
    IDENT = mybir.ActivationFunctionType.Identity

    nc = bacc.Bacc()

    xT = nc.dram_tensor("xT", [C, N], bf16, kind="ExternalInput")
    wkq = nc.dram_tensor("wkq", [C, 2 * C], bf16, kind="ExternalInput")
    wv = nc.dram_tensor("wv", [C, C], bf16, kind="ExternalInput")
    prmw = nc.dram_tensor("prmw", [128, 3 * 2 * 128], f32, kind="ExternalInput")
    sqw = nc.dram_tensor("sqw", [128, 2 * 128], f32, kind="ExternalInput")
    zf = nc.dram_tensor("zf", [128, H * 32], bf16, kind="ExternalInput")
    kqb = nc.dram_tensor("kqb", [128, 12], f32, kind="ExternalInput")
    vbr = nc.dram_tensor("vbr", [128, C], f32, kind="ExternalInput")
    pwT = nc.dram_tensor("pwT", [C, C], bf16, kind="ExternalInput")
    pb = nc.dram_tensor("pb", [128, 6], f32, kind="ExternalInput")
    bcsel = nc.dram_tensor("bcsel", [68, 3 * 128], bf16, kind="ExternalInput")
    identf = nc.dram_tensor("identf", [128, 128], f32, kind="ExternalInput")
    yT = nc.dram_tensor("yT", [C, N], f32, kind="ExternalOutput")

    with tile.TileContext(nc) as tc:
        import contextlib
        with contextlib.ExitStack() as ctx:
            const = ctx.enter_context(tc.tile_pool(name="const", bufs=1))

            # ---- resident constants ----
            # DMA order matters: one FIFO queue, HBM-bound preload gates the
            # first matmul. Interleave xt(tile0) chunks with wkq chunks so
            # the kq chains can start as chunks land; everything not needed
            # for tile0's kq blocks loads afterwards.
            wkq_sb = []
            wv_sb = []
            xt0 = const.tile([128, 6, T], bf16, tag="xt0")
            for c in range(6):
                t_ = const.tile([128, 2 * C], bf16, tag=f"wkq{c}")
                nc.sync.dma_start(t_, wkq[c * 128:(c + 1) * 128, :])
                wkq_sb.append(t_)
                nc.sync.dma_start(
                    xt0[:, c, :], xT[c * 128:(c + 1) * 128, 0:T])
            kqb_sb = const.tile([128, 12], f32, tag="kqb")
            nc.sync.dma_start(kqb_sb, kqb[:])
            prmw_sb = const.tile([128, 3, 2, 128], f32r, tag="prmw")
            nc.sync.dma_start(prmw_sb, prmw[:].bitcast(f32r).rearrange(
                "p (g s c) -> p g s c", g=3, s=2))
            sqw_sb = const.tile([128, 2, 128], f32r, tag="sqw")
            nc.sync.dma_start(sqw_sb, sqw[:].bitcast(f32r).rearrange(
                "p (s c) -> p s c", s=2))
            for c in range(6):
                t_ = const.tile([128, C], bf16, tag=f"wv{c}")
                nc.sync.dma_start(t_, wv[c * 128:(c + 1) * 128, :])
                wv_sb.append(t_)
            vbr_sb = const.tile([128, C], f32, tag="vbr")
            nc.sync.dma_start(vbr_sb, vbr[:])
            identf_sb = const.tile([128, 128], f32, tag="identf")
            nc.sync.dma_start(identf_sb, identf[:])
            bcsel_sb = const.tile([68, 3, 128], bf16, tag="bcsel")
            nc.sync.dma_start(bcsel_sb, bcsel[:].rearrange(
                "p (g c) -> p g c", g=3))

            # resident accumulators / state
            qp_pack = const.tile([128, 3, N], bf16, tag="qp_pack")
            acc = const.tile([128, 3, M * 2 + 1], f32, tag="acc")
            nc.vector.memset(acc, 0.0)


            # acc layout: [128, g, 65]; head h = 4g+gi lives at partitions
            # 32gi..32gi+32: cols 0:64 = kptv^T, col 64 = kp_sum

            # ================= PASS 1 =================
            with tc.tile_pool(name="xt", bufs=3) as xtp, \
                 tc.tile_pool(name="pp", bufs=3) as ppool, \
                 tc.tile_pool(name="bp", bufs=3) as bpool, \
                 tc.tile_pool(name="kpt", bufs=3) as kptp, \
                 tc.tile_pool(name="kpn", bufs=4) as kpnp, \
                 tc.tile_pool(name="vaug", bufs=6) as vap, \
                 tc.tile_pool(name="ps_kq", bufs=3, space="PSUM") as ps_kq, \
                 tc.tile_pool(name="ps_v", bufs=1, space="PSUM") as ps_v, \
                 tc.tile_pool(name="ps_prm", bufs=2, space="PSUM") as ps_prm, \
                 tc.tile_pool(name="ps_kptv", bufs=1, space="PSUM") as ps_kptv:

                for it in range(NT):
                    t0 = it * T
                    if it == 0:
                        xt = xt0
                    else:
                        xt = xtp.tile([128, 6, T], bf16, tag="xt")
                        for c in range(6):
                            nc.sync.dma_start(
                                xt[:, c, :],
                                xT[c * 128:(c + 1) * 128, t0:t0 + T])

                    vaugs = []

                    def emit_v(sub):
                        psv = ps_v.tile([128, C], f32, tag="psv")
                        for c in range(6):
                            lhs = xt[:, c, sub * 128:(sub + 1) * 128]
                            nc.tensor.matmul(psv[:, 0:512], lhs,
                                             wv_sb[c][:, 0:512],
                                             start=(c == 0), stop=(c == 5))
                            nc.tensor.matmul(psv[:, 512:768], lhs,
                                             wv_sb[c][:, 512:768],
                                             start=(c == 0), stop=(c == 5))
                        va = vap.tile([128, H, HD + 1], bf16, tag="vaug")
                        nc.vector.tensor_tensor(
                            va[:, :, 0:HD],
                            psv[:].rearrange("p (h n) -> p h n", h=H),
                            vbr_sb[:].rearrange("p (h n) -> p h n", h=H), ADD)
                        nc.vector.memset(va[:, :, HD:HD + 1], 1.0)
                        vaugs.append(va)

                    # ---- pair blocks: per group g, k-pairs then q-pairs ----
                    # block order: [k2g, k2g+1, q2g, q2g+1] for g in 0..2
                    kpts = []
                    nblk = 0
                    for g in range(3):
                        for kind in range(2):      # 0 = k, 1 = q
                            pg = ps_prm.tile([128, T], f32, tag="prm")
                            for half in range(2):  # pair within the group
                                blk = kind * 6 + 2 * g + half
                                if nblk % 3 == 2:
                                    emit_v(nblk // 3)
                                nblk += 1
                                pkq = ps_kq.tile([128, T], f32, tag="pkq")
                                for c in range(6):
                                    nc.tensor.matmul(
                                        pkq,
                                        wkq_sb[c][:, blk * 128:(blk + 1) * 128],
                                        xt[:, c, :],
                                        start=(c == 0), stop=(c == 5))
                                bias = kqb_sb[:, blk:blk + 1]
                                P = ppool.tile([128, T], f32r, tag="pp")
                                Bt = bpool.tile([128, T], f32r, tag="bp")
                                if blk % 2 == 0:
                                    nc.scalar.activation(P, pkq, IDENT,
                                                         bias=bias)
                                    nc.vector.scalar_tensor_tensor(
                                        Bt, pkq, bias, P, ADD, MULT)
                                else:
                                    nc.vector.tensor_scalar_add(P, pkq, bias)
                                    nc.scalar.activation(Bt, pkq, SQUARE,
                                                         bias=bias)
                                nc.tensor.matmul(
                                    pg, prmw_sb[:, g, half, :], P,
                                    start=(half == 0), stop=False)
                                nc.tensor.matmul(
                                    pg, sqw_sb[:, half, :], Bt,
                                    start=False, stop=(half == 1))
                            if kind == 0:
                                kpt = kptp.tile([128, T], bf16, tag="kpt")
                                nc.scalar.activation(kpt, pg, EXP)
                                kpts.append(kpt)
                            else:
                                nc.scalar.activation(
                                    qp_pack[:, g, t0:t0 + T], pg, EXP)

                    # ---- kp transpose + kptv accumulation ----
                    pkptv = ps_kptv.tile([128, 3, HD + 1], f32, tag="pkptv")
                    for g in range(3):
                        for j in range(4):
                            kpn = kpnp.tile([128, 128], bf16, tag="kpn")
                            nc.scalar.dma_start_transpose(
                                kpn, kpts[g][:, j * 128:(j + 1) * 128])
                            for gi in range(4):
                                h = g * 4 + gi
                                nc.tensor.matmul(
                                    pkptv[32 * gi:32 * (gi + 1), g, :],
                                    kpn[:, 32 * gi:32 * (gi + 1)],
                                    vaugs[j][:, h, :],
                                    start=(j == 0), stop=(j == 3),
                                    tile_position=(0, 32 * gi))
                    for g in range(3):
                        nc.vector.tensor_tensor(acc[:, g, :], pkptv[:, g, :],
                                                acc[:, g, :], ADD)

            # ================= PASS 2 =================
            pwT_sb = []
            for c in range(6):
                t_ = const.tile([128, C], bf16, tag=f"pwT{c}")
                nc.sync.dma_start(t_, pwT[c * 128:(c + 1) * 128, :])
                pwT_sb.append(t_)
            pb_sb = const.tile([128, 6], f32, tag="pb")
            nc.sync.dma_start(pb_sb, pb[:])

            # ---- once: G = kptv^T-fold into proj, D lhsT from kp_sum ----
            G_sb = const.tile([128, 3, C], bf16, tag="G")
            Dlhs = const.tile([128, 3, 4], bf16, tag="Dlhs")
            nc.vector.memset(Dlhs, 0.0)
            Glhs = const.tile([128, H, 32], bf16, tag="Glhs")
            nc.sync.dma_start(Glhs, zf[:].rearrange("p (h c) -> p h c", h=H))
            eps_sb = const.tile([128, 1], f32, tag="eps")
            nc.vector.memset(eps_sb, EPS_EFF)

            with tc.tile_pool(name="rc", bufs=2) as rcp, \
                 tc.tile_pool(name="qpn", bufs=4) as qpnp, \
                 tc.tile_pool(name="so", bufs=4) as sop, \
                 tc.tile_pool(name="ps_d", bufs=1, space="PSUM") as ps_d, \
                 tc.tile_pool(name="ps_bc", bufs=2, space="PSUM") as ps_bc:

                for g in range(3):
                    for gi in range(4):
                        nc.vector.tensor_copy(
                            Dlhs[32 * gi:32 * (gi + 1), g, gi:gi + 1],
                            acc[32 * gi:32 * (gi + 1), g, HD:HD + 1])

                def emit_qpn(t0, tl):
                    dps = ps_d.tile([128, T], f32, tag="dps")
                    for g in range(3):
                        nc.tensor.matmul(
                            dps[32 * g:32 * g + 4, 0:tl], Dlhs[:, g, :],
                            qp_pack[:, g, t0:t0 + tl],
                            start=True, stop=True,
                            tile_position=(0, 32 * g))
                    rc12 = rcp.tile([68, T], f32, tag="rc12")
                    nc.vector.memset(rc12[:, 0:tl], 1.0)
                    for g in range(3):
                        nc.scalar.activation(
                            rc12[32 * g:32 * g + 4, 0:tl],
                            dps[32 * g:32 * g + 4, 0:tl], IDENT,
                            bias=eps_sb[0:4, :])
                    nc.vector.reciprocal_approx_fast(out=rc12[:, 0:tl],
                                                     in_=rc12[:, 0:tl])
                    rc12r = rcp.tile([68, T], bf16, tag="rc12r")
                    nc.scalar.activation(rc12r[:, 0:tl], rc12[:, 0:tl], IDENT)
                    qpns = []
                    for g in range(3):
                        bcg = ps_bc.tile([128, T], f32, tag="bcg")
                        nc.tensor.matmul(bcg[:, 0:tl], bcsel_sb[:, g, :],
                                         rc12r[:, 0:tl],
                                         start=True, stop=True)
                        qpn = qpnp.tile([128, T], bf16, tag="qpn")
                        nc.vector.tensor_tensor(
                            qpn[:, 0:tl], qp_pack[:, g, t0:t0 + tl],
                            bcg[:, 0:tl], MULT)
                        qpns.append(qpn)
                    return qpns

                # tile-0 qpn prologue overlaps the G fold below
                qpns0 = emit_qpn(0, T)

                gctx = contextlib.ExitStack()
                ps_trg = gctx.enter_context(
                    tc.tile_pool(name="ps_trg", bufs=1, space="PSUM"))
                ps_g = gctx.enter_context(
                    tc.tile_pool(name="ps_g", bufs=1, space="PSUM"))
                for g in range(3):
                    ptr = ps_trg.tile([64, 128], f32, tag="ptrg")
                    nc.tensor.transpose(ptr, acc[:, g, 0:HD], identf_sb)
                    for gi in range(4):
                        h = 4 * g + gi
                        nc.vector.tensor_copy(
                            Glhs[64 * (h % 2):64 * (h % 2 + 1), h, :],
                            ptr[:, 32 * gi:32 * (gi + 1)])
                for g in range(3):
                    gps = ps_g.tile([128, C], f32, tag="gps")
                    for gi in range(4):
                        h = 4 * g + gi
                        cchunk = h // 2
                        for o0, o1 in ((0, 512), (512, 768)):
                            nc.tensor.matmul(
                                gps[32 * gi:32 * (gi + 1), o0:o1],
                                Glhs[:, h, :],
                                pwT_sb[cchunk][:, o0:o1],
                                start=True, stop=True,
                                tile_position=(0, 32 * gi))
                    nc.scalar.activation(G_sb[:, g, :], gps, IDENT)
                gctx.close()

                with tc.tile_pool(name="ps_f", bufs=3, space="PSUM") as ps_f:
                    for t0, tl in [(i * T, T) for i in range(NT)]:
                        qpns = qpns0 if t0 == 0 else emit_qpn(t0, tl)

                        for o2 in range(6):
                            pf = ps_f.tile([128, T], f32, tag="pf")
                            for g in range(3):
                                nc.tensor.matmul(
                                    pf[:, 0:tl],
                                    G_sb[:, g, o2 * 128:(o2 + 1) * 128],
                                    qpns[g][:, 0:tl],
                                    start=(g == 0), stop=(g == 2))
                            so = sop.tile([128, T], f32, tag="so")
                            if o2 % 2 == 0:
                                nc.scalar.activation(so[:, 0:tl], pf[:, 0:tl],
                                                     IDENT,
                                                     bias=pb_sb[:, o2:o2 + 1])
                            else:
                                nc.vector.tensor_scalar_add(
                                    so[:, 0:tl], pf[:, 0:tl],
                                    pb_sb[:, o2:o2 + 1])
                            nc.sync.dma_start(yT[o2 * 128:(o2 + 1) * 128,
                                                 t0:t0 + tl], so[:, 0:tl])

    nc.compile()
    return nc


def _prep_inputs(x, kqv_w, kqv_b, proj_w, proj_b, w):
    x = np.asarray(x, np.float32)
    kqv_w = np.asarray(kqv_w, np.float32)
    kqv_b = np.asarray(kqv_b, np.float32)
    proj_w = np.asarray(proj_w, np.float32)
    proj_b = np.asarray(proj_b, np.float32)
    w = np.asarray(w, np.float32)

    Wk, Wq = kqv_w[0:C], kqv_w[C:2 * C]
    Wv = kqv_w[2 * C:3 * C]
    # 12 blocks of 128 cols: blocks 0..5 = k-pairs (2b, 2b+1), 6..11 = q-pairs
    wkq = np.empty((C, 2 * C), np.float32)  # cast to bf16 below
    for b in range(6):
        wkq[:, b * 128:b * 128 + 64] = Wk[2 * b * 64:(2 * b + 1) * 64, :].T
        wkq[:, b * 128 + 64:(b + 1) * 128] = Wk[(2 * b + 1) * 64:
                                                (2 * b + 2) * 64, :].T
        qb = 6 + b
        wkq[:, qb * 128:qb * 128 + 64] = Wq[2 * b * 64:(2 * b + 1) * 64, :].T
        wkq[:, qb * 128 + 64:(qb + 1) * 128] = Wq[(2 * b + 1) * 64:
                                                  (2 * b + 2) * 64, :].T
    wv = np.ascontiguousarray(Wv.T).astype(ml_dtypes.bfloat16)

    # block-diagonal Performer weights: [128, group, half, 128]; half 0
    # covers psum partitions 0:64 (heads 4g,4g+1), half 1 covers 64:128
    prmw = np.zeros((128, 3, 2, 128), np.float32)
    for g in range(3):
        prmw[0:64, g, 0, 0:32] = w[4 * g].T
        prmw[64:128, g, 0, 32:64] = w[4 * g + 1].T
        prmw[0:64, g, 1, 64:96] = w[4 * g + 2].T
        prmw[64:128, g, 1, 96:128] = w[4 * g + 3].T
    prmw = prmw.reshape(128, 3 * 2 * 128)
    sqw = np.zeros((128, 2, 128), np.float32)
    sqw[0:64, 0, 0:32] = -0.5
    sqw[64:128, 0, 32:64] = -0.5
    sqw[0:64, 1, 64:96] = -0.5
    sqw[64:128, 1, 96:128] = -0.5
    sqw = sqw.reshape(128, 2 * 128)
    zf = np.zeros((128, H * 32), ml_dtypes.bfloat16)

    # pair biases: col b = [bias_h0(64); bias_h1(64)]
    kqb = np.zeros((128, 12), np.float32)
    for b in range(6):
        kqb[0:64, b] = kqv_b[2 * b * 64:(2 * b + 1) * 64]
        kqb[64:128, b] = kqv_b[(2 * b + 1) * 64:(2 * b + 2) * 64]
        kqb[0:64, 6 + b] = kqv_b[C + 2 * b * 64:C + (2 * b + 1) * 64]
        kqb[64:128, 6 + b] = kqv_b[C + (2 * b + 1) * 64:C + (2 * b + 2) * 64]
    vbr = np.broadcast_to(kqv_b[2 * C:3 * C], (128, C)).copy()

    pwT = np.ascontiguousarray(proj_w.T).astype(ml_dtypes.bfloat16)
    pb = np.ascontiguousarray(proj_b.reshape(6, 128).T)

    # broadcast selector: row r=32g+gi of group g -> cols 32gi:32(gi+1)
    bcsel = np.zeros((68, 3 * 128), ml_dtypes.bfloat16)
    for g in range(3):
        for gi in range(4):
            bcsel[32 * g + gi, g * 128 + 32 * gi:g * 128 + 32 * (gi + 1)] = 1.0

    identf = np.eye(128, dtype=np.float32)

    shared = {"wkq": wkq.astype(ml_dtypes.bfloat16), "wv": wv, "prmw": prmw, "sqw": sqw, "zf": zf,
              "kqb": kqb, "vbr": vbr, "pwT": pwT, "pb": pb, "bcsel": bcsel,
              "identf": identf}
    xTb = np.ascontiguousarray(
        x.transpose(0, 2, 1)).astype(ml_dtypes.bfloat16)  # [B, C, N]
    return [dict(shared, xT=xTb[b]) for b in range(B)]


def kernel(x, kqv_w, kqv_b, proj_w, proj_b, w):
    global LAST_EXEC_NS
    from concourse.bass_utils import run_bass_kernel_spmd

    if "nc" not in _CACHE:
        _CACHE["nc"] = _build()
    nc = _CACHE["nc"]

    in_maps = _prep_inputs(x, kqv_w, kqv_b, proj_w, proj_b, w)
    res = run_bass_kernel_spmd(nc, in_maps, list(range(B)), trace=TRACE)
    LAST_EXEC_NS = res.exec_time_ns
    out = np.empty((B, N, C), np.float32)
    for b in range(B):
        out[b] = res.results[b]["yT"].T
    return out


# revision 49
# speedup vs baseline: 1.4194x; 1.4194x over previous
"""AttentionPerformer Trainium2 kernel.

Strategy: data-parallel over batch B=8 -> one NeuronCore per batch element.
Feature-major (transposed) layout throughout; Performer random-feature
exponent (wtx - 0.5*|k|^2) comes out of the PE via block-diagonal
pair-packed contractions so every elementwise op runs on all 128 partitions.

Math (per b, h; eps folded: dropping both 1/sqrt(M) factors rescales
numerator and denominator by M, so eps becomes M*eps):
    kq^T = W_kq^T @ x^T        (feature-major; head-PAIR blocks [k_h0|k_h1])
    P = kq + b ; B = (kq+b)^2  (full 128-partition ops per pair block)
    e  = blockdiag(w_h0,w_h1)^T P + blockdiag(-.5,-.5)^T B
         (two accumulating matmuls; two pairs pack a [128,T] psum -> 1 EXP
          yields kp/qp for a 4-head group in kptv layout directly)
    kptv[m, n] = sum_t kp[t,m] v[t,n] ;  kp_sum[m] = sum_t kp[t,m]
    G_h = kptv_h @ proj_w_h^T  (fold kptv into proj: [M, C] per head, once)
    D = kp_sum . qp  (packed 4-head matmuls) ; qpn = qp * 1/(D+M*eps)
         (reciprocal broadcast back to 128 partitions via a selector matmul)
    out^T = sum_g G_g^T @ qpn_g + proj_b   (contraction 3x128 instead of 768)

dtypes: fp32r on the exponent-critical path, bf16 on the kp/qp/v/G value
path (errors average over the M=32 contraction).
"""
import sys
sys.path.insert(0, '/opt/trn_rl_repo')

import numpy as np
import ml_dtypes

B, N, C = 8, 4096, 768
H, HD, M = 12, 64, 32
T = 512                 # t-tile size
NT = N // T             # 8 tiles
EPS_EFF = float(M) * 1e-8

_CACHE = {}
TRACE = False
LAST_EXEC_NS = None


def _build():
    import concourse.bass as bass
    import concourse.tile as tile
    from concourse import bacc, mybir

    f32 = mybir.dt.float32
    f32r = mybir.dt.float32r
    bf16 = mybir.dt.bfloat16
    ADD = mybir.AluOpType.add
    MULT = mybir.AluOpType.mult
    EXP = mybir.ActivationFunctionType.Exp
    SQUARE = mybir.ActivationFunctionType.Square
    IDENT = mybir.ActivationFunctionType.Identity

    nc = bacc.Bacc()

    xT = nc.dram_tensor("xT", [C, N], bf16, kind="ExternalInput")
    wkq = nc.dram_tensor("wkq", [C, 2 * C], bf16, kind="ExternalInput")
    wv = nc.dram_tensor("wv", [C, C], bf16, kind="ExternalInput")
    prmw = nc.dram_tensor("prmw", [128, 3 * 2 * 128], f32, kind="ExternalInput")
    sqw = nc.dram_tensor("sqw", [128, 2 * 128], f32, kind="ExternalInput")
    zf = nc.dram_tensor("zf", [128, H * 32], bf16, kind="ExternalInput")
    kqb = nc.dram_tensor("kqb", [128, 12], f32, kind="ExternalInput")
    vbr = nc.dram_tensor("vbr", [128, C], f32, kind="ExternalInput")
    pwT = nc.dram_tensor("pwT", [C, C], bf16, kind="ExternalInput")
    pb = nc.dram_tensor("pb", [128, 6], f32, kind="ExternalInput")
    bcsel = nc.dram_tensor("bcsel", [68, 3 * 128], bf16, kind="ExternalInput")
    identb = nc.dram_tensor("identb", [128, 128], bf16, kind="ExternalInput")
    identf = nc.dram_tensor("identf", [128, 128], f32, kind="ExternalInput")
    yT = nc.dram_tensor("yT", [C, N], f32, kind="ExternalOutput")

    with tile.TileContext(nc) as tc:
        import contextlib
        with contextlib.ExitStack() as ctx:
            const = ctx.enter_context(tc.tile_pool(name="const", bufs=1))

            # ---- resident constants ----
            # DMA order matters: one FIFO queue, HBM-bound preload gates the
            # first matmul. Interleave xt(tile0) chunks with wkq chunks so
            # the kq chains can start as chunks land; everything not needed
            # for tile0's kq blocks loads afterwards.
            wkq_sb = []
            wv_sb = []
            xt0 = const.tile([128, 6, T], bf16, tag="xt0")
            for c in range(6):
                t_ = const.tile([128, 2 * C], bf16, tag=f"wkq{c}")
                nc.sync.dma_start(t_, wkq[c * 128:(c + 1) * 128, :])
                wkq_sb.append(t_)
                nc.sync.dma_start(
                    xt0[:, c, :], xT[c * 128:(c + 1) * 128, 0:T])
            kqb_sb = const.tile([128, 12], f32, tag="kqb")
            nc.sync.dma_start(kqb_sb, kqb[:])
            prmw_sb = const.tile([128, 3, 2, 128], f32r, tag="prmw")
            nc.sync.dma_start(prmw_sb, prmw[:].bitcast(f32r).rearrange(
                "p (g s c) -> p g s c", g=3, s=2))
            sqw_sb = const.tile([128, 2, 128], f32r, tag="sqw")
            nc.sync.dma_start(sqw_sb, sqw[:].bitcast(f32r).rearrange(
                "p (s c) -> p s c", s=2))
            for c in range(6):
                t_ = const.tile([128, C], bf16, tag=f"wv{c}")
                nc.sync.dma_start(t_, wv[c * 128:(c + 1) * 128, :])
                wv_sb.append(t_)
            vbr_sb = const.tile([128, C], f32, tag="vbr")
            nc.sync.dma_start(vbr_sb, vbr[:])
            ident_sb = const.tile([128, 128], bf16, tag="identb")
            nc.sync.dma_start(ident_sb, identb[:])
            identf_sb = const.tile([128, 128], f32, tag="identf")
            nc.sync.dma_start(identf_sb, identf[:])
            bcsel_sb = const.tile([68, 3, 128], bf16, tag="bcsel")
            nc.sync.dma_start(bcsel_sb, bcsel[:].rearrange(
                "p (g c) -> p g c", g=3))

            # resident accumulators / state
            qp_pack = const.tile([128, 3, N], bf16, tag="qp_pack")
            acc = const.tile([128, 3, M * 2 + 1], f32, tag="acc")
            nc.vector.memset(acc, 0.0)


            # acc layout: [128, g, 65]; head h = 4g+gi lives at partitions
            # 32gi..32gi+32: cols 0:64 = kptv^T, col 64 = kp_sum

            # ================= PASS 1 =================
            with tc.tile_pool(name="xt", bufs=3) as xtp, \
                 tc.tile_pool(name="pp", bufs=3) as ppool, \
                 tc.tile_pool(name="bp", bufs=3) as bpool, \
                 tc.tile_pool(name="kpt", bufs=3) as kptp, \
                 tc.tile_pool(name="kpn", bufs=4) as kpnp, \
                 tc.tile_pool(name="vaug", bufs=6) as vap, \
                 tc.tile_pool(name="ps_kq", bufs=2, space="PSUM") as ps_kq, \
                 tc.tile_pool(name="ps_v", bufs=1, space="PSUM") as ps_v, \
                 tc.tile_pool(name="ps_prm", bufs=2, space="PSUM") as ps_prm, \
                 tc.tile_pool(name="ps_kptv", bufs=1, space="PSUM") as ps_kptv, \
                 tc.tile_pool(name="ps_tr", bufs=1, space="PSUM") as ps_tr:

                for it in range(NT):
                    t0 = it * T
                    if it == 0:
                        xt = xt0
                    else:
                        xt = xtp.tile([128, 6, T], bf16, tag="xt")
                        for c in range(6):
                            nc.sync.dma_start(
                                xt[:, c, :],
                                xT[c * 128:(c + 1) * 128, t0:t0 + T])

                    vaugs = []

                    def emit_v(sub):
                        psv = ps_v.tile([128, C], f32, tag="psv")
                        for c in range(6):
                            lhs = xt[:, c, sub * 128:(sub + 1) * 128]
                            nc.tensor.matmul(psv[:, 0:512], lhs,
                                             wv_sb[c][:, 0:512],
                                             start=(c == 0), stop=(c == 5))
                            nc.tensor.matmul(psv[:, 512:768], lhs,
                                             wv_sb[c][:, 512:768],
                                             start=(c == 0), stop=(c == 5))
                        va = vap.tile([128, H, HD + 1], bf16, tag="vaug")
                        nc.vector.tensor_tensor(
                            va[:, :, 0:HD],
                            psv[:].rearrange("p (h n) -> p h n", h=H),
                            vbr_sb[:].rearrange("p (h n) -> p h n", h=H), ADD)
                        nc.vector.memset(va[:, :, HD:HD + 1], 1.0)
                        vaugs.append(va)

                    # ---- pair blocks: per group g, k-pairs then q-pairs ----
                    # block order: [k2g, k2g+1, q2g, q2g+1] for g in 0..2
                    kpts = []
                    nblk = 0
                    for g in range(3):
                        for kind in range(2):      # 0 = k, 1 = q
                            pg = ps_prm.tile([128, T], f32, tag="prm")
                            for half in range(2):  # pair within the group
                                blk = kind * 6 + 2 * g + half
                                if nblk % 3 == 2:
                                    emit_v(nblk // 3)
                                nblk += 1
                                pkq = ps_kq.tile([128, T], f32, tag="pkq")
                                for c in range(6):
                                    nc.tensor.matmul(
                                        pkq,
                                        wkq_sb[c][:, blk * 128:(blk + 1) * 128],
                                        xt[:, c, :],
                                        start=(c == 0), stop=(c == 5))
                                bias = kqb_sb[:, blk:blk + 1]
                                P = ppool.tile([128, T], f32r, tag="pp")
                                Bt = bpool.tile([128, T], f32r, tag="bp")
                                if blk % 2 == 0:
                                    nc.scalar.activation(P, pkq, IDENT,
                                                         bias=bias)
                                    nc.vector.scalar_tensor_tensor(
                                        Bt, pkq, bias, P, ADD, MULT)
                                else:
                                    nc.vector.tensor_scalar_add(P, pkq, bias)
                                    nc.scalar.activation(Bt, pkq, SQUARE,
                                                         bias=bias)
                                nc.tensor.matmul(
                                    pg, prmw_sb[:, g, half, :], P,
                                    start=(half == 0), stop=False)
                                nc.tensor.matmul(
                                    pg, sqw_sb[:, half, :], Bt,
                                    start=False, stop=(half == 1))
                            if kind == 0:
                                kpt = kptp.tile([128, T], bf16, tag="kpt")
                                nc.scalar.activation(kpt, pg, EXP)
                                kpts.append(kpt)
                            else:
                                nc.scalar.activation(
                                    qp_pack[:, g, t0:t0 + T], pg, EXP)

                    # ---- kp transpose + kptv accumulation ----
                    pkptv = ps_kptv.tile([128, 3, HD + 1], f32, tag="pkptv")
                    for g in range(3):
                        for j in range(4):
                            ptr = ps_tr.tile([128, 128], bf16, tag="ptr")
                            nc.tensor.transpose(
                                ptr, kpts[g][:, j * 128:(j + 1) * 128],
                                ident_sb)
                            kpn = kpnp.tile([128, 128], bf16, tag="kpn")
                            nc.vector.tensor_copy(kpn, ptr)
                            for gi in range(4):
                                h = g * 4 + gi
                                nc.tensor.matmul(
                                    pkptv[32 * gi:32 * (gi + 1), g, :],
                                    kpn[:, 32 * gi:32 * (gi + 1)],
                                    vaugs[j][:, h, :],
                                    start=(j == 0), stop=(j == 3),
                                    tile_position=(0, 32 * gi))
                    for g in range(3):
                        nc.vector.tensor_tensor(acc[:, g, :], pkptv[:, g, :],
                                                acc[:, g, :], ADD)

            # ================= PASS 2 =================
            pwT_sb = []
            for c in range(6):
                t_ = const.tile([128, C], bf16, tag=f"pwT{c}")
                nc.sync.dma_start(t_, pwT[c * 128:(c + 1) * 128, :])
                pwT_sb.append(t_)
            pb_sb = const.tile([128, 6], f32, tag="pb")
            nc.sync.dma_start(pb_sb, pb[:])

            # ---- once: G = kptv^T-fold into proj, D lhsT from kp_sum ----
            G_sb = const.tile([128, 3, C], bf16, tag="G")
            Dlhs = const.tile([128, 3, 4], bf16, tag="Dlhs")
            nc.vector.memset(Dlhs, 0.0)
            Glhs = const.tile([128, H, 32], bf16, tag="Glhs")
            nc.sync.dma_start(Glhs, zf[:].rearrange("p (h c) -> p h c", h=H))
            eps_sb = const.tile([128, 1], f32, tag="eps")
            nc.vector.memset(eps_sb, EPS_EFF)

            with tc.tile_pool(name="rc", bufs=2) as rcp, \
                 tc.tile_pool(name="qpn", bufs=4) as qpnp, \
                 tc.tile_pool(name="so", bufs=4) as sop, \
                 tc.tile_pool(name="ps_d", bufs=1, space="PSUM") as ps_d, \
                 tc.tile_pool(name="ps_bc", bufs=2, space="PSUM") as ps_bc:

                for g in range(3):
                    for gi in range(4):
                        nc.vector.tensor_copy(
                            Dlhs[32 * gi:32 * (gi + 1), g, gi:gi + 1],
                            acc[32 * gi:32 * (gi + 1), g, HD:HD + 1])

                def emit_qpn(t0, tl):
                    dps = ps_d.tile([128, T], f32, tag="dps")
                    for g in range(3):
                        nc.tensor.matmul(
                            dps[32 * g:32 * g + 4, 0:tl], Dlhs[:, g, :],
                            qp_pack[:, g, t0:t0 + tl],
                            start=True, stop=True,
                            tile_position=(0, 32 * g))
                    rc12 = rcp.tile([68, T], f32, tag="rc12")
                    nc.vector.memset(rc12[:, 0:tl], 1.0)
                    for g in range(3):
                        nc.scalar.activation(
                            rc12[32 * g:32 * g + 4, 0:tl],
                            dps[32 * g:32 * g + 4, 0:tl], IDENT,
                            bias=eps_sb[0:4, :])
                    nc.vector.reciprocal_approx_fast(out=rc12[:, 0:tl],
                                                     in_=rc12[:, 0:tl])
                    rc12r = rcp.tile([68, T], bf16, tag="rc12r")
                    nc.scalar.activation(rc12r[:, 0:tl], rc12[:, 0:tl], IDENT)
                    qpns = []
                    for g in range(3):
                        bcg = ps_bc.tile([128, T], f32, tag="bcg")
                        nc.tensor.matmul(bcg[:, 0:tl], bcsel_sb[:, g, :],
                                         rc12r[:, 0:tl],
                                         start=True, stop=True)
                        qpn = qpnp.tile([128, T], bf16, tag="qpn")
                        nc.vector.tensor_tensor(
                            qpn[:, 0:tl], qp_pack[:, g, t0:t0 + tl],
                            bcg[:, 0:tl], MULT)
                        qpns.append(qpn)
                    return qpns

                # tile-0 qpn prologue overlaps the G fold below
                qpns0 = emit_qpn(0, T)

                gctx = contextlib.ExitStack()
                ps_trg = gctx.enter_context(
                    tc.tile_pool(name="ps_trg", bufs=1, space="PSUM"))
                ps_g = gctx.enter_context(
                    tc.tile_pool(name="ps_g", bufs=1, space="PSUM"))
                for g in range(3):
                    ptr = ps_trg.tile([64, 128], f32, tag="ptrg")
                    nc.tensor.transpose(ptr, acc[:, g, 0:HD], identf_sb)
                    for gi in range(4):
                        h = 4 * g + gi
                        nc.vector.tensor_copy(
                            Glhs[64 * (h % 2):64 * (h % 2 + 1), h, :],
                            ptr[:, 32 * gi:32 * (gi + 1)])
                for g in range(3):
                    gps = ps_g.tile([128, C], f32, tag="gps")
                    for gi in range(4):
                        h = 4 * g + gi
                        cchunk = h // 2
                        for o0, o1 in ((0, 512), (512, 768)):
                            nc.tensor.matmul(
                                gps[32 * gi:32 * (gi + 1), o0:o1],
                                Glhs[:, h, :],
                                pwT_sb[cchunk][:, o0:o1],
                                start=True, stop=True,
                                tile_position=(0, 32 * gi))
                    nc.scalar.activation(G_sb[:, g, :], gps, IDENT)
                gctx.close()

                with tc.tile_pool(name="ps_f", bufs=3, space="PSUM") as ps_f:
                    for t0, tl in [(i * T, T) for i in range(NT)]:
                        qpns = qpns0 if t0 == 0 else emit_qpn(t0, tl)

                        for o2 in range(6):
                            pf = ps_f.tile([128, T], f32, tag="pf")
                            for g in range(3):
                                nc.tensor.matmul(
                                    pf[:, 0:tl],
                                    G_sb[:, g, o2 * 128:(o2 + 1) * 128],
                                    qpns[g][:, 0:tl],
                                    start=(g == 0), stop=(g == 2))
                            so = sop.tile([128, T], f32, tag="so")
                            if o2 % 2 == 0:
                                nc.scalar.activation(so[:, 0:tl], pf[:, 0:tl],
                                                     IDENT,
                                                     bias=pb_sb[:, o2:o2 + 1])
                            else:
                                nc.vector.tensor_scalar_add(
                                    so[:, 0:tl], pf[:, 0:tl],
                                    pb_sb[:, o2:o2 + 1])
                            nc.sync.dma_start(yT[o2 * 128:(o2 + 1) * 128,
                                                 t0:t0 + tl], so[:, 0:tl])

    nc.compile()
    return nc


def _prep_inputs(x, kqv_w, kqv_b, proj_w, proj_b, w):
    x = np.asarray(x, np.float32)
    kqv_w = np.asarray(kqv_w, np.float32)
    kqv_b = np.asarray(kqv_b, np.float32)
    proj_w = np.asarray(proj_w, np.float32)
    proj_b = np.asarray(proj_b, np.float32)
    w = np.asarray(w, np.float32)

    Wk, Wq = kqv_w[0:C], kqv_w[C:2 * C]
    Wv = kqv_w[2 * C:3 * C]
    # 12 blocks of 128 cols: blocks 0..5 = k-pairs (2b, 2b+1), 6..11 = q-pairs
    wkq = np.empty((C, 2 * C), np.float32)  # cast to bf16 below
    for b in range(6):
        wkq[:, b * 128:b * 128 + 64] = Wk[2 * b * 64:(2 * b + 1) * 64, :].T
        wkq[:, b * 128 + 64:(b + 1) * 128] = Wk[(2 * b + 1) * 64:
                                                (2 * b + 2) * 64, :].T
        qb = 6 + b
        wkq[:, qb * 128:qb * 128 + 64] = Wq[2 * b * 64:(2 * b + 1) * 64, :].T
        wkq[:, qb * 128 + 64:(qb + 1) * 128] = Wq[(2 * b + 1) * 64:
                                                  (2 * b + 2) * 64, :].T
    wv = np.ascontiguousarray(Wv.T).astype(ml_dtypes.bfloat16)

    # block-diagonal Performer weights: [128, group, half, 128]; half 0
    # covers psum partitions 0:64 (heads 4g,4g+1), half 1 covers 64:128
    prmw = np.zeros((128, 3, 2, 128), np.float32)
    for g in range(3):
        prmw[0:64, g, 0, 0:32] = w[4 * g].T
        prmw[64:128, g, 0, 32:64] = w[4 * g + 1].T
        prmw[0:64, g, 1, 64:96] = w[4 * g + 2].T
        prmw[64:128, g, 1, 96:128] = w[4 * g + 3].T
    prmw = prmw.reshape(128, 3 * 2 * 128)
    sqw = np.zeros((128, 2, 128), np.float32)
    sqw[0:64, 0, 0:32] = -0.5
    sqw[64:128, 0, 32:64] = -0.5
    sqw[0:64, 1, 64:96] = -0.5
    sqw[64:128, 1, 96:128] = -0.5
    sqw = sqw.reshape(128, 2 * 128)
    zf = np.zeros((128, H * 32), ml_dtypes.bfloat16)

    # pair biases: col b = [bias_h0(64); bias_h1(64)]
    kqb = np.zeros((128, 12), np.float32)
    for b in range(6):
        kqb[0:64, b] = kqv_b[2 * b * 64:(2 * b + 1) * 64]
        kqb[64:128, b] = kqv_b[(2 * b + 1) * 64:(2 * b + 2) * 64]
        kqb[0:64, 6 + b] = kqv_b[C + 2 * b * 64:C + (2 * b + 1) * 64]
        kqb[64:128, 6 + b] = kqv_b[C + (2 * b + 1) * 64:C + (2 * b + 2) * 64]
    vbr = np.broadcast_to(kqv_b[2 * C:3 * C], (128, C)).copy()

    pwT = np.ascontiguousarray(proj_w.T).astype(ml_dtypes.bfloat16)
    pb = np.ascontiguousarray(proj_b.reshape(6, 128).T)

    # broadcast selector: row r=32g+gi of group g -> cols 32gi:32(gi+1)
    bcsel = np.zeros((68, 3 * 128), ml_dtypes.bfloat16)
    for g in range(3):
        for gi in range(4):
            bcsel[32 * g + gi, g * 128 + 32 * gi:g * 128 + 32 * (gi + 1)] = 1.0

    identb = np.eye(128, dtype=ml_dtypes.bfloat16)
    identf = np.eye(128, dtype=np.float32)

    shared = {"wkq": wkq.astype(ml_dtypes.bfloat16), "wv": wv, "prmw": prmw, "sqw": sqw, "zf": zf,
              "kqb": kqb, "vbr": vbr, "pwT": pwT, "pb": pb, "bcsel": bcsel,
              "identb": identb, "identf": identf}
    xTb = np.ascontiguousarray(
        x.transpose(0, 2, 1)).astype(ml_dtypes.bfloat16)  # [B, C, N]
    return [dict(shared, xT=xTb[b]) for b in range(B)]


def kernel(x, kqv_w, kqv_b, proj_w, proj_b, w):
    global LAST_EXEC_NS
    from concourse.bass_utils import run_bass_kernel_spmd

    if "nc" not in _CACHE:
        _CACHE["nc"] = _build()
    nc = _CACHE["nc"]

    in_maps = _prep_inputs(x, kqv_w, kqv_b, proj_w, proj_b, w)
    res = run_bass_kernel_spmd(nc, in_maps, list(range(B)), trace=TRACE)
    LAST_EXEC_NS = res.exec_time_ns
    out = np.empty((B, N, C), np.float32)
    for b in range(B):
        out[b] = res.results[b]["yT"].T
    return out
